# revision 10
# baseline (speedup 1.0000x reference)
"""Trainium2 kernel for nn_ButterflyProduct.

The module applies, 10 times, a weighted (softmax) sum of 10 butterfly
factors to the last dim of x.  Every step is a linear operator on the
1024-dim axis, so the whole forward pass collapses to a single
1024x1024 matrix W applied to x:

    out = x @ W,   W = (M_0 @ M_1 @ ... @ M_9)^T,
    M_i = sum_j softmax(logit)[i,j] * B_j

W is composed on the host (float64) from the tiny parameter tensors and
the 17.2 GFLOP batch application runs data-parallel across 8
NeuronCores: each core computes a [1024,1024] @ [1024,1024] matmul for
its batch shard.

The host also pre-transposes each x shard (contraction dim onto
partitions) and casts both operands to fp16: the PE streams fp16 at the
same 1 row/cycle as fp32r but with half the HBM traffic, no transpose
phase, and none of the fp32 power mode.  fp16 quantization error for
these inputs is ~4e-4 max-rel; returning fp16 outputs adds ~5e-4 more.

Device kernel (per core, fully unrolled Tile program).  TRN2 has two
hardware DGE queues (SP and Activation); a single queue moves only
~240 GB/s, which starves the PE, so the streams are split:
  sync  queue: W k-row-blocks in, then all out row-blocks
  scalar queue: xt k-row-blocks in
  half A (batch rows 0..511):  8 PSUM accumulators (4 row-blocks x 2
          column chunks), k outermost -- chunk k is consumed right as
          it lands (k=0 tiles are DMA'd in halves so the PE starts
          ~0.5us earlier).
  half B (rows 512..1023): inputs all resident, k-inner + n-split per
          row-block so each 128-row output DMA streams behind the next
          block's matmuls and the final tail is one evac + 256 KiB.
"""

import numpy as np
from contextlib import ExitStack

import concourse.bass as bass
import concourse.bacc as bacc
import concourse.mybir as mybir
import concourse.tile as tile
from concourse.bass_utils import run_bass_kernel_spmd

SIZE = 1024
M = 10
N_TERMS = 10
BATCH = 8192
NCORES = 8
SHARD = BATCH // NCORES  # 1024
DIAGS = [1 << (M - 1 - j) for j in range(M)]

P = 128
NB = SHARD // P       # 8 batch row-blocks per core
NK = SIZE // P        # 8 contraction tiles
NFREE = 512           # matmul moving free dim (one psum bank)
NN = SIZE // NFREE    # 2 output column chunks

IN_DT = mybir.dt.float16
IN_NP = np.float16


def _compose_w(diag, subpad, suppad, logit):
    """Compose the full linear operator W (float64) so out = x @ W."""
    lg = logit.astype(np.float64)
    e = np.exp(lg - lg.max(axis=-1, keepdims=True))
    prob = e / e.sum(axis=-1, keepdims=True)          # (N_TERMS, M)
    dg = diag.astype(np.float64)
    sb = subpad.astype(np.float64)
    sp = suppad.astype(np.float64)

    A = np.eye(SIZE, dtype=np.float64)
    for i in range(N_TERMS)[::-1]:
        D = (prob[i][:, None] * dg).sum(0)            # combined diagonal
        out = D[:, None] * A
        for j in range(M):
            d = DIAGS[j]
            out[d:] += (prob[i, j] * sb[j, d:])[:, None] * A[:-d]
            out[:-d] += (prob[i, j] * sp[j, :-d])[:, None] * A[d:]
        A = out                                       # A = M_i @ ... @ M_9
    return np.ascontiguousarray(A.T.astype(np.float32))


def _slim_drain_and_barrier(self, tick_clock, wait_clock):
    """Replacement for TileContext._drain_and_barrier: keep the sync-engine
    drain that waits for every queue/engine tick (this is what guarantees the
    output DMAs have landed), drop the two all-engine barriers and the
    semaphore clears — the Bass preamble re-clears all semaphores at the next
    execution's start, so end-of-kernel hygiene costs ~7us for nothing."""
    from concourse.tile import ScopedClock

    drain_inst = self.nc.sync.drain()
    wait_clock.add_sem_waits(
        drain_inst.ins, ScopedClock({None: tick_clock.global_clock})
    )
    popped = self.nc._tile_sem_poison_stack.pop()
    assert popped is self._sem_poison


def _build_program():
    # Bacc (not raw Bass): its finalize() pipeline splits semaphore waits
    # (move_matmul_waits_to_ldweights / generate_event_semaphores) to meet
    # the 1-wait-per-instruction hardware limit walrus enforces.
    nc = bacc.Bacc(None, target_bir_lowering=False)
    xt = nc.dram_tensor("xt", [SIZE, SHARD], IN_DT, kind="ExternalInput")
    w = nc.dram_tensor("w", [SIZE, SIZE], IN_DT, kind="ExternalInput")
    out = nc.dram_tensor("out", [SHARD, SIZE], IN_DT, kind="ExternalOutput")

    orig_dab = tile.TileContext._drain_and_barrier
    tile.TileContext._drain_and_barrier = _slim_drain_and_barrier
    try:
        _emit_body(nc, xt, w, out)
    finally:
        tile.TileContext._drain_and_barrier = orig_dab

    nc.finalize()
    return nc


def _emit_body(nc, xt, w, out):
    f32 = mybir.dt.float32

    # Warm-up DMAs, emitted in the entry block BEFORE the tile context:
    # they issue during the entry barrier / library-load window, so the
    # DMA engines and HBM path are already spun up when the first real
    # W/xt transfers are issued — those otherwise pay ~1-2us of cold
    # ramp that delays the first matmul.
    warm_w = nc.alloc_sbuf_tensor("warm_w", [16, SIZE], IN_DT)
    warm_x = nc.alloc_sbuf_tensor("warm_x", [16, SHARD], IN_DT)
    warm_sem = nc.alloc_semaphore("warm_sem")
    nc.sync.dma_start(warm_w.ap(), w[0:16, :]).then_inc(warm_sem, 16)
    nc.scalar.dma_start(warm_x.ap(), xt[0:16, :]).then_inc(warm_sem, 16)

    with ExitStack() as ctx:
        tc = ctx.enter_context(tile.TileContext(nc))
        wpool = ctx.enter_context(tc.tile_pool(name="wpool", bufs=1))
        xtpool = ctx.enter_context(tc.tile_pool(name="xtpool", bufs=1))
        opool = ctx.enter_context(tc.tile_pool(name="opool", bufs=4))
        psum = ctx.enter_context(tc.tile_pool(name="psum", bufs=8, space="PSUM"))

        w_all = wpool.tile([P, NK * SIZE], IN_DT, tag="w")
        xt_all = xtpool.tile([P, NK * SHARD], IN_DT, tag="xt")

        # Parallel feed on the two HW queues, k ascending.  k=0 in two
        # halves so the first matmul group's operands land sooner.
        half = SIZE // 2
        for h in range(2):
            nc.sync.dma_start(
                w_all[:, h * half:(h + 1) * half],
                w[0:P, h * half:(h + 1) * half])
            nc.scalar.dma_start(
                xt_all[:, h * half:(h + 1) * half],
                xt[0:P, h * half:(h + 1) * half])
        for k in range(1, NK):
            nc.sync.dma_start(
                w_all[:, k * SIZE:(k + 1) * SIZE], w[k * P:(k + 1) * P, :])
            nc.scalar.dma_start(
                xt_all[:, k * SHARD:(k + 1) * SHARD], xt[k * P:(k + 1) * P, :])

        def w_sb(k, n):
            return w_all[:, k * SIZE + n * NFREE:k * SIZE + (n + 1) * NFREE]

        def xt_sb(k, i):
            return xt_all[:, k * SHARD + i * P:k * SHARD + (i + 1) * P]

        # All evacs on DVE: using nc.scalar.copy would pull in an ACT
        # table load on the Activation engine, which delays that engine's
        # first xt DMA issue (and through the entry barrier, everything
        # else) by ~1.3us.
        def evac_and_store(i, accs):
            ot = opool.tile([P, SIZE], IN_DT, tag="ot")
            for n in range(NN):
                nc.vector.tensor_copy(
                    ot[:, n * NFREE:(n + 1) * NFREE], accs[n][:])
            nc.sync.dma_start(out[i * P:(i + 1) * P, :], ot[:])

        # ── half A: batch row-blocks 0..3, k outermost over 8 open
        # accumulators, so chunk k is consumed right as it arrives ──
        NIA = NB // 2
        accs = {}
        for i in range(NIA):
            for n in range(NN):
                accs[(i, n)] = psum.tile([P, NFREE], f32,
                                         tag="ps", name=f"accA_{i}_{n}")
        for k in range(NK):
            for i in range(NIA):
                for n in range(NN):
                    nc.tensor.matmul(
                        accs[(i, n)][:],
                        xt_sb(k, i),
                        w_sb(k, n),
                        start=(k == 0),
                        stop=(k == NK - 1),
                    )
        for i in range(NIA):
            evac_and_store(i, [accs[(i, n)] for n in range(NN)])

        # ── half B: row-blocks 4..7, everything resident; k-inner,
        # n-split per block, and a separate out-DMA per n-chunk so the
        # n=0 chunk's store overlaps the n=1 matmuls.  The last block
        # uses 4 chunks of 256 so the unoverlappable tail is only one
        # 256-col evac + one 64 KiB transfer ──
        for i in range(NIA, NB):
            ot = opool.tile([P, SIZE], IN_DT, tag="ot")
            nfree = NFREE if i < NB - 1 else NFREE // 2
            for n in range(SIZE // nfree):
                acc = psum.tile([P, nfree], f32, tag="ps", name=f"accB_{i}_{n}")
                for k in range(NK):
                    nc.tensor.matmul(
                        acc[:],
                        xt_sb(k, i),
                        w_all[:, k * SIZE + n * nfree:
                              k * SIZE + (n + 1) * nfree],
                        start=(k == 0),
                        stop=(k == NK - 1),
                    )
                nc.vector.tensor_copy(
                    ot[:, n * nfree:(n + 1) * nfree], acc[:])
                nc.sync.dma_start(
                    out[i * P:(i + 1) * P, n * nfree:(n + 1) * nfree],
                    ot[:, n * nfree:(n + 1) * nfree])


_prog = None


def _make_in_maps(x, diag, subpad, suppad, logit):
    W = _compose_w(np.asarray(diag), np.asarray(subpad),
                   np.asarray(suppad), np.asarray(logit)).astype(IN_NP)
    x = np.asarray(x, dtype=np.float32)
    xs = x.reshape(NCORES, SHARD, SIZE)
    return [
        {"xt": np.ascontiguousarray(xs[c].T.astype(IN_NP)), "w": W}
        for c in range(NCORES)
    ]


def kernel(x, diag, subpad, suppad, logit):
    global _prog
    in_maps = _make_in_maps(x, diag, subpad, suppad, logit)
    if _prog is None:
        _prog = _build_program()
    res = run_bass_kernel_spmd(_prog, in_maps, list(range(NCORES)))
    return np.concatenate(
        [r["out"].astype(np.float32) for r in res.results], axis=0)


# revision 11
# speedup vs baseline: 1.0430x; 1.0430x over previous
"""Trainium2 kernel for nn_ButterflyProduct.

The module applies, 10 times, a weighted (softmax) sum of 10 butterfly
factors to the last dim of x.  Every step is a linear operator on the
1024-dim axis, so the whole forward pass collapses to a single
1024x1024 matrix W applied to x:

    out = x @ W,   W = (M_0 @ M_1 @ ... @ M_9)^T,
    M_i = sum_j softmax(logit)[i,j] * B_j

W is composed on the host (float64) from the tiny parameter tensors and
the 17.2 GFLOP batch application runs data-parallel across 8
NeuronCores: each core computes a [1024,1024] @ [1024,1024] matmul for
its batch shard.

The host also pre-transposes each x shard (contraction dim onto
partitions) and casts both operands to fp16: the PE streams fp16 at the
same 1 row/cycle as fp32r but with half the HBM traffic, no transpose
phase, and none of the fp32 power mode.  fp16 quantization error for
these inputs is ~4e-4 max-rel; returning fp16 outputs adds ~5e-4 more.

Device kernel (per core, fully unrolled Tile program).  TRN2 has two
hardware DGE queues (SP and Activation); a single queue moves only
~240 GB/s, which starves the PE, so the streams are split:
  sync  queue: W k-row-blocks in, then all out row-blocks
  scalar queue: xt k-row-blocks in
  half A (batch rows 0..511):  8 PSUM accumulators (4 row-blocks x 2
          column chunks), k outermost -- chunk k is consumed right as
          it lands (k=0 tiles are DMA'd in halves so the PE starts
          ~0.5us earlier).
  half B (rows 512..1023): inputs all resident, k-inner + n-split per
          row-block so each 128-row output DMA streams behind the next
          block's matmuls and the final tail is one evac + 256 KiB.
"""

import numpy as np
from contextlib import ExitStack

import concourse.bass as bass
import concourse.bacc as bacc
import concourse.mybir as mybir
import concourse.tile as tile
from concourse.bass_utils import run_bass_kernel_spmd

SIZE = 1024
M = 10
N_TERMS = 10
BATCH = 8192
NCORES = 8
SHARD = BATCH // NCORES  # 1024
DIAGS = [1 << (M - 1 - j) for j in range(M)]

P = 128
NB = SHARD // P       # 8 batch row-blocks per core
NK = SIZE // P        # 8 contraction tiles
NFREE = 512           # matmul moving free dim (one psum bank)
NN = SIZE // NFREE    # 2 output column chunks

IN_DT = mybir.dt.float16
IN_NP = np.float16


def _compose_w(diag, subpad, suppad, logit):
    """Compose the full linear operator W (float64) so out = x @ W."""
    lg = logit.astype(np.float64)
    e = np.exp(lg - lg.max(axis=-1, keepdims=True))
    prob = e / e.sum(axis=-1, keepdims=True)          # (N_TERMS, M)
    dg = diag.astype(np.float64)
    sb = subpad.astype(np.float64)
    sp = suppad.astype(np.float64)

    A = np.eye(SIZE, dtype=np.float64)
    for i in range(N_TERMS)[::-1]:
        D = (prob[i][:, None] * dg).sum(0)            # combined diagonal
        out = D[:, None] * A
        for j in range(M):
            d = DIAGS[j]
            out[d:] += (prob[i, j] * sb[j, d:])[:, None] * A[:-d]
            out[:-d] += (prob[i, j] * sp[j, :-d])[:, None] * A[d:]
        A = out                                       # A = M_i @ ... @ M_9
    return np.ascontiguousarray(A.T.astype(np.float32))


def _slim_drain_and_barrier(self, tick_clock, wait_clock):
    """Replacement for TileContext._drain_and_barrier: keep the sync-engine
    drain that waits for every queue/engine tick (this is what guarantees the
    output DMAs have landed), drop the two all-engine barriers and the
    semaphore clears — the Bass preamble re-clears all semaphores at the next
    execution's start, so end-of-kernel hygiene costs ~7us for nothing."""
    from concourse.tile import ScopedClock

    drain_inst = self.nc.sync.drain()
    wait_clock.add_sem_waits(
        drain_inst.ins, ScopedClock({None: tick_clock.global_clock})
    )
    popped = self.nc._tile_sem_poison_stack.pop()
    assert popped is self._sem_poison


def _build_program():
    # Bacc (not raw Bass): its finalize() pipeline splits semaphore waits
    # (move_matmul_waits_to_ldweights / generate_event_semaphores) to meet
    # the 1-wait-per-instruction hardware limit walrus enforces.
    nc = bacc.Bacc(None, target_bir_lowering=False)
    xt = nc.dram_tensor("xt", [SIZE, SHARD], IN_DT, kind="ExternalInput")
    w = nc.dram_tensor("w", [SIZE, SIZE], IN_DT, kind="ExternalInput")
    out = nc.dram_tensor("out", [SHARD, SIZE], IN_DT, kind="ExternalOutput")

    orig_dab = tile.TileContext._drain_and_barrier
    tile.TileContext._drain_and_barrier = _slim_drain_and_barrier
    try:
        _emit_body(nc, xt, w, out)
    finally:
        tile.TileContext._drain_and_barrier = orig_dab

    nc.finalize()
    return nc


def _emit_body(nc, xt, w, out):
    f32 = mybir.dt.float32

    with ExitStack() as ctx:
        tc = ctx.enter_context(tile.TileContext(nc))
        wpool = ctx.enter_context(tc.tile_pool(name="wpool", bufs=1))
        xtpool = ctx.enter_context(tc.tile_pool(name="xtpool", bufs=1))
        opool = ctx.enter_context(tc.tile_pool(name="opool", bufs=4))
        psum = ctx.enter_context(tc.tile_pool(name="psum", bufs=8, space="PSUM"))

        w_all = wpool.tile([P, NK * SIZE], IN_DT, tag="w")
        xt_all = xtpool.tile([P, NK * SHARD], IN_DT, tag="xt")

        # Parallel feed on the two HW queues, k ascending.  k=0 in two
        # halves so the first matmul group's operands land sooner.
        half = SIZE // 2
        for h in range(2):
            nc.sync.dma_start(
                w_all[:, h * half:(h + 1) * half],
                w[0:P, h * half:(h + 1) * half])
            nc.scalar.dma_start(
                xt_all[:, h * half:(h + 1) * half],
                xt[0:P, h * half:(h + 1) * half])
        for k in range(1, NK):
            nc.sync.dma_start(
                w_all[:, k * SIZE:(k + 1) * SIZE], w[k * P:(k + 1) * P, :])
            nc.scalar.dma_start(
                xt_all[:, k * SHARD:(k + 1) * SHARD], xt[k * P:(k + 1) * P, :])

        def w_sb(k, n):
            return w_all[:, k * SIZE + n * NFREE:k * SIZE + (n + 1) * NFREE]

        def xt_sb(k, i):
            return xt_all[:, k * SHARD + i * P:k * SHARD + (i + 1) * P]

        # All evacs on DVE: using nc.scalar.copy would pull in an ACT
        # table load on the Activation engine, which delays that engine's
        # first xt DMA issue (and through the entry barrier, everything
        # else) by ~1.3us.
        def evac_and_store(i, accs):
            ot = opool.tile([P, SIZE], IN_DT, tag="ot")
            for n in range(NN):
                nc.vector.tensor_copy(
                    ot[:, n * NFREE:(n + 1) * NFREE], accs[n][:])
            nc.sync.dma_start(out[i * P:(i + 1) * P, :], ot[:])

        # ── half A: batch row-blocks 0..3, k outermost over 8 open
        # accumulators, so chunk k is consumed right as it arrives ──
        NIA = NB // 2
        accs = {}
        for i in range(NIA):
            for n in range(NN):
                accs[(i, n)] = psum.tile([P, NFREE], f32,
                                         tag="ps", name=f"accA_{i}_{n}")
        for k in range(NK):
            for i in range(NIA):
                for n in range(NN):
                    nc.tensor.matmul(
                        accs[(i, n)][:],
                        xt_sb(k, i),
                        w_sb(k, n),
                        start=(k == 0),
                        stop=(k == NK - 1),
                    )
        for i in range(NIA):
            evac_and_store(i, [accs[(i, n)] for n in range(NN)])

        # ── half B: row-blocks 4..7, everything resident; k-inner,
        # n-split per block, and a separate out-DMA per n-chunk so the
        # n=0 chunk's store overlaps the n=1 matmuls.  The last block
        # uses 4 chunks of 256 so the unoverlappable tail is only one
        # 256-col evac + one 64 KiB transfer ──
        for i in range(NIA, NB):
            ot = opool.tile([P, SIZE], IN_DT, tag="ot")
            nfree = NFREE if i < NB - 1 else NFREE // 2
            for n in range(SIZE // nfree):
                acc = psum.tile([P, nfree], f32, tag="ps", name=f"accB_{i}_{n}")
                for k in range(NK):
                    nc.tensor.matmul(
                        acc[:],
                        xt_sb(k, i),
                        w_all[:, k * SIZE + n * nfree:
                              k * SIZE + (n + 1) * nfree],
                        start=(k == 0),
                        stop=(k == NK - 1),
                    )
                nc.vector.tensor_copy(
                    ot[:, n * nfree:(n + 1) * nfree], acc[:])
                nc.sync.dma_start(
                    out[i * P:(i + 1) * P, n * nfree:(n + 1) * nfree],
                    ot[:, n * nfree:(n + 1) * nfree])


_prog = None


def _make_in_maps(x, diag, subpad, suppad, logit):
    W = _compose_w(np.asarray(diag), np.asarray(subpad),
                   np.asarray(suppad), np.asarray(logit)).astype(IN_NP)
    x = np.asarray(x, dtype=np.float32)
    xs = x.reshape(NCORES, SHARD, SIZE)
    return [
        {"xt": np.ascontiguousarray(xs[c].T.astype(IN_NP)), "w": W}
        for c in range(NCORES)
    ]


def kernel(x, diag, subpad, suppad, logit):
    global _prog
    in_maps = _make_in_maps(x, diag, subpad, suppad, logit)
    if _prog is None:
        _prog = _build_program()
    res = run_bass_kernel_spmd(_prog, in_maps, list(range(NCORES)))
    return np.concatenate(
        [r["out"].astype(np.float32) for r in res.results], axis=0)
